# revision 3
# baseline (speedup 1.0000x reference)
"""GRU cell kernel for Trainium2, 8-core data-parallel, single dispatch,
partial-fp8 matmuls.

Strategy
--------
Data-parallel on batch across 8 cores; each core processes its full
2048-row shard in ONE dispatch.  All on-chip compute happens in
transposed space ([hidden, batch]):

    r^T = sigmoid(W_r @ x^T + U_r @ h^T + b_r)      <- fp8 DoubleRow
    u^T = sigmoid(W_u @ x^T + U_u @ h^T + b_u)      <- bf16
    c^T = tanh   (W @ x^T   + U @ (h.r)^T + b_c)    <- W@x bf16, U@hr fp8
    o^T = h^T + u^T * (c^T - h^T)

fp8e4(DoubleRow) matmuls run 2x the bf16 rate on real HW (measured).
Full-fp8 busts the 2e-2 error budget (3.2e-2 measured on the real
inputs), but the reset-gate path is attenuated by sigmoid' and U-row
norms (rel 4.0e-3) and the U@hr matmul by tanh'/u (combined rel
1.2e-2), so exactly those run in fp8.

Measured-HW tensor-engine facts baked into the schedule:
  - ldweights serializes with the matmul (~44ns per 128-row stationary);
    processing the two chunks of a PAIR back-to-back under one
    stationary halves it.  fp8 DoubleRow weight loads are ~hidden.
  - Each DMA instruction occupies the single HWDGE generator ~625ns
    regardless of size -> consolidated loads (one per (matrix, out-tile)
    and per x/h chunk; 65 loads total).

Weights + x/h are fully SBUF-resident in distinct tile slots, so no DMA
ever writes a recycled slot -- this toolchain's DMA descriptors encode
exactly ONE sync wait, so any DMA needing a cross-engine WAR/RAW wait
on top of its queue-FIFO wait fails walrus codegen.  Loads carry only
queue waits; output stores go out via the Pool-engine SWDGE queue,
spaced several us apart so their single RAW wait fits.  Biases ride the
ScalarE activation (per-partition bias) which also evicts PSUM and
casts in the same instruction.

Per-partition SBUF budget (usable ~208 KiB):
  weights 48(bf16)+24(fp8) + x 32+16 + h 32+16 + hr8 8 + u 16
  + r/c/out ~11 = ~203 KiB.
"""

import sys

sys.path.insert(0, "/opt/trn_rl_repo")

import numpy as np
import ml_dtypes
from contextlib import ExitStack

import concourse.bass as bass
import concourse.bacc as bacc
import concourse.mybir as mybir
from concourse import tile
from concourse.bass_utils import run_bass_kernel_spmd

BF16 = mybir.dt.bfloat16
FP8 = mybir.dt.float8e4
F32 = mybir.dt.float32
AF = mybir.ActivationFunctionType
DR = mybir.MatmulPerfMode.DoubleRow

N_CORES = 8
B = 16384
D = 1024  # IN == H
N_ROUNDS = 1
B_SHARD = B // N_CORES  # 2048 rows per core
W = 512  # chunk width (one fp32 PSUM bank)
PAIR = 2  # chunks per stationary-reuse group


def build_nc(d=D, b_shard=B_SHARD, w=W):
    """Build the SPMD per-core Bass program.

    wts8 packs fp8 mats [W_r, U_r, U]; wtsb packs bf16 mats [W_u, U_u, W].
    Bias columns: [r: 0..nh) [u: nh..2nh) [c: 2nh..3nh).
    """
    nk = d // 128
    nh = d // 128
    nch = b_shard // w

    nc = bacc.Bacc("TRN2", target_bir_lowering=False)
    xt = nc.dram_tensor("xt", [d, b_shard], BF16, kind="ExternalInput")
    ht = nc.dram_tensor("ht", [d, b_shard], BF16, kind="ExternalInput")
    xt8 = nc.dram_tensor("xt8", [d, b_shard], FP8, kind="ExternalInput")
    ht8 = nc.dram_tensor("ht8", [d, b_shard], FP8, kind="ExternalInput")
    # wts*[mat, j, p, k*128+m] = M.T[k*128+p, j*128+m]
    wtsb = nc.dram_tensor("wtsb", [3, nh, 128, nk * 128], BF16, kind="ExternalInput")
    wts8 = nc.dram_tensor("wts8", [3, nh, 128, nk * 128], FP8, kind="ExternalInput")
    bias = nc.dram_tensor("bias", [128, 3 * nh], F32, kind="ExternalInput")
    out = nc.dram_tensor("out", [d, b_shard], F32, kind="ExternalOutput")

    with tile.TileContext(nc) as tc, ExitStack() as ctx:
        xp = ctx.enter_context(tc.tile_pool(name="xp", bufs=nch))
        hp = ctx.enter_context(tc.tile_pool(name="hp", bufs=nch))
        xp8 = ctx.enter_context(tc.tile_pool(name="xp8", bufs=nch))
        hp8 = ctx.enter_context(tc.tile_pool(name="hp8", bufs=nch))
        up = ctx.enter_context(tc.tile_pool(name="up", bufs=PAIR * nh))
        hr8p = ctx.enter_context(tc.tile_pool(name="hr8p", bufs=PAIR))
        cp = ctx.enter_context(tc.tile_pool(name="cp", bufs=3))
        rp = ctx.enter_context(tc.tile_pool(name="rp", bufs=2))
        wpb = ctx.enter_context(tc.tile_pool(name="wpb", bufs=3 * nh))
        wp8 = ctx.enter_context(tc.tile_pool(name="wp8", bufs=3 * nh))
        bp = ctx.enter_context(tc.tile_pool(name="bp", bufs=1))
        op = ctx.enter_context(tc.tile_pool(name="op", bufs=3))
        pp = ctx.enter_context(tc.tile_pool(name="pp", bufs=8, space="PSUM"))

        xts = [None] * nch  # [chunk] -> [128, nk, w] bf16
        hts = [None] * nch
        x8s = [None] * nch  # [chunk] -> [128, nk, w] fp8
        h8s = [None] * nch
        wb = [[None] * nh for _ in range(3)]  # [W_u, U_u, W] bf16 [128, nk, 128]
        w8 = [[None] * nh for _ in range(3)]  # [W_r, U_r, U] fp8 [128, nk, 128]

        def load_chunk(pool, src, store, c, dt):
            t = pool.tile([128, nk, w], dt, name=f"{src.name}tile")
            nc.sync.dma_start(
                t, src[:, c * w : (c + 1) * w].rearrange("(k p) n -> p k n", p=128)
            )
            store[c] = t

        def load_w(src, store, mat, j, dt):
            t = (wp8 if dt is FP8 else wpb).tile([128, nk, 128], dt, name="wtile")
            nc.sync.dma_start(t, src[mat, j, :, :].rearrange("p (k m) -> p k m", m=128))
            store[mat][j] = t

        # DMA issue order = consumption order.  R pair-0 needs only the fp8
        # weights + pair-0 fp8 x/h; U pair-0 (starting ~30us in) needs bf16
        # x/h + Wu/Uu, so those come right after; C-phase weights and the
        # second pair's chunks trail.
        # j0 of the R phase consumes BOTH chunks of pair 0 (pair-interleaved
        # matmuls), x-part first: x8 for both chunks, then Ur0/h8.
        load_w(wts8, w8, 0, 0, FP8)
        for c in range(PAIR):
            load_chunk(xp8, xt8, x8s, c, FP8)
        load_w(wts8, w8, 0, 1, FP8)
        for c in range(PAIR):
            load_chunk(hp8, ht8, h8s, c, FP8)
        load_w(wts8, w8, 1, 0, FP8)
        load_w(wts8, w8, 1, 1, FP8)
        btile = bp.tile([128, 3 * nh], F32, name="btile")
        nc.sync.dma_start(btile, bias[:, :])
        for j in range(2, nh):
            load_w(wts8, w8, 0, j, FP8)
            load_w(wts8, w8, 1, j, FP8)
        for c in range(PAIR):
            load_chunk(xp, xt, xts, c, BF16)
            load_chunk(hp, ht, hts, c, BF16)
        for j in range(nh):
            load_w(wtsb, wb, 0, j, BF16)
            load_w(wtsb, wb, 1, j, BF16)
        for j in range(nh):
            load_w(wtsb, wb, 2, j, BF16)
        for j in range(nh):
            load_w(wts8, w8, 2, j, FP8)
        for c in range(PAIR, nch):
            load_chunk(xp8, xt8, x8s, c, FP8)
            load_chunk(hp8, ht8, h8s, c, FP8)
            load_chunk(xp, xt, xts, c, BF16)
            load_chunk(hp, ht, hts, c, BF16)

        def pair_matmuls_bf16(cs, j, mat_x, mov_x, mat_h, mov_h):
            """One PSUM bank per chunk of the pair; stationary reused across
            the pair's chunks (halves the serialized ldweights cost)."""
            pss = [pp.tile([128, w], F32, name="ps") for _ in cs]
            for mi, (mat, movs) in enumerate(((mat_x, mov_x), (mat_h, mov_h))):
                for k in range(nk):
                    lhsT = wb[mat][j][:, k, :]
                    for ci, c in enumerate(cs):
                        nc.tensor.matmul(
                            pss[ci], lhsT, movs[c][:, k, :],
                            start=(mi == 0 and k == 0),
                            stop=(mi == 1 and k == nk - 1),
                        )
            return pss

        for p in range(nch // PAIR):
            cs = list(range(p * PAIR, (p + 1) * PAIR))

            # R phase (fp8 DoubleRow): r = sigmoid(.); hr8 = fp8(h * r)
            hr8 = {}
            for c in cs:
                hr8[c] = hr8p.tile([128, nk, w], FP8, name="hr8tile")

            def r_mats(pss, j, mat, movs, is_start, is_stop):
                for kk in range(0, nk, 2):
                    lhsT = w8[mat][j][:, kk : kk + 2, :]
                    for ci, c in enumerate(cs):
                        nc.tensor.matmul(
                            pss[ci], lhsT, movs[c][:, kk : kk + 2, :],
                            start=(is_start and kk == 0),
                            stop=(is_stop and kk == nk - 2),
                            perf_mode=DR,
                        )

            def r_tail(pss, j):
                for ci, c in enumerate(cs):
                    rtile = rp.tile([128, w], BF16, name="rtile")
                    nc.scalar.activation(
                        rtile, pss[ci], AF.Sigmoid, bias=btile[:, j : j + 1]
                    )
                    nc.vector.tensor_mul(hr8[c][:, j, :], hts[c][:, j, :], rtile)

            if p == 0:
                # warm-up: run j0+j1 x-parts back to back so the h8 chunk
                # loads (queued behind x8) land while the PE is busy
                ps01 = {}
                for j in (0, 1):
                    ps01[j] = [pp.tile([128, w], F32, name="ps") for _ in cs]
                    r_mats(ps01[j], j, 0, x8s, True, False)
                for j in (0, 1):
                    r_mats(ps01[j], j, 1, h8s, False, True)
                    r_tail(ps01[j], j)
                j_start = 2
            else:
                j_start = 0
            for j in range(j_start, nh):
                pss = [pp.tile([128, w], F32, name="ps") for _ in cs]
                r_mats(pss, j, 0, x8s, True, False)
                r_mats(pss, j, 1, h8s, False, True)
                r_tail(pss, j)

            # U phase (bf16)
            us = {}
            for j in range(nh):
                pss = pair_matmuls_bf16(cs, j, 0, xts, 1, hts)
                for ci, c in enumerate(cs):
                    util = up.tile([128, w], BF16, name="utile")
                    nc.scalar.activation(
                        util, pss[ci], AF.Sigmoid, bias=btile[:, nh + j : nh + j + 1]
                    )
                    us[(c, j)] = util

            # C phase: W@x in bf16, U@hr in fp8 DoubleRow; OUT fused per (j,c)
            for j in range(nh):
                pss = [pp.tile([128, w], F32, name="ps") for _ in cs]
                for k in range(nk):
                    lhsT = wb[2][j][:, k, :]
                    for ci, c in enumerate(cs):
                        nc.tensor.matmul(
                            pss[ci], lhsT, xts[c][:, k, :],
                            start=(k == 0), stop=False,
                        )
                for kk in range(0, nk, 2):
                    lhsT = w8[2][j][:, kk : kk + 2, :]
                    for ci, c in enumerate(cs):
                        nc.tensor.matmul(
                            pss[ci], lhsT, hr8[c][:, kk : kk + 2, :],
                            start=False, stop=(kk == nk - 2),
                            perf_mode=DR,
                        )
                for ci, c in enumerate(cs):
                    ctile = cp.tile([128, w], BF16, name="ctile")
                    nc.scalar.activation(
                        ctile, pss[ci], AF.Tanh, bias=btile[:, 2 * nh + j : 2 * nh + j + 1]
                    )
                    tb = cp.tile([128, w], BF16, name="tbtile")
                    nc.vector.tensor_sub(tb, ctile, hts[c][:, j, :])
                    nc.vector.tensor_mul(tb, us[(c, j)], tb)
                    t = op.tile([128, w], F32, name="ttile")
                    nc.vector.tensor_add(t, tb, hts[c][:, j, :])
                    # store via the Activation engine's HWDGE queue: single
                    # RAW wait on t (loads use the SP queue, so no FIFO
                    # backpressure), avoiding the Pool SWDGE's ~1us
                    # descriptor-generation that serialized the tail.
                    nc.scalar.dma_start(
                        out[j * 128 : (j + 1) * 128, c * w : (c + 1) * w], t
                    )

    nc.compile()
    return nc


def pack_inputs(inputs, d=D, b_shard=B_SHARD, n_shards=N_CORES):
    """Host-side shard + transpose + cast. Returns per-shard input maps."""
    nk = d // 128
    nh = d // 128
    x = np.asarray(inputs["x_t"], np.float32)
    h = np.asarray(inputs["h_prev"], np.float32)

    def pack_w(m, dt):
        mt = np.asarray(m, np.float32).T.astype(dt)  # [in, out]
        # [j, p, k*128+m']
        return mt.reshape(nk, 128, nh, 128).transpose(2, 1, 0, 3).reshape(nh, 128, nk * 128)

    wtsb = np.stack(
        [pack_w(inputs[nm], ml_dtypes.bfloat16) for nm in ("W_u", "U_u", "W")]
    )
    wts8 = np.stack(
        [pack_w(inputs[nm], ml_dtypes.float8_e4m3fn) for nm in ("W_r", "U_r", "U")]
    )

    b_r = np.asarray(inputs["b_Wr"], np.float32) + np.asarray(inputs["b_Ur"], np.float32)
    b_u = np.asarray(inputs["b_Wu"], np.float32) + np.asarray(inputs["b_Uu"], np.float32)
    b_c = np.asarray(inputs["b_W"], np.float32) + np.asarray(inputs["b_U"], np.float32)
    bias = np.concatenate(
        [bb.reshape(nh, 128).T for bb in (b_r, b_u, b_c)], axis=1
    ).astype(np.float32)  # [128, 3*nh]

    in_maps = []
    for s in range(n_shards):
        rows = slice(s * b_shard, (s + 1) * b_shard)
        xT = np.ascontiguousarray(x[rows].T)
        hT = np.ascontiguousarray(h[rows].T)
        in_maps.append(
            {
                "xt": xT.astype(ml_dtypes.bfloat16),
                "ht": hT.astype(ml_dtypes.bfloat16),
                "xt8": xT.astype(ml_dtypes.float8_e4m3fn),
                "ht8": hT.astype(ml_dtypes.float8_e4m3fn),
                "wtsb": wtsb,
                "wts8": wts8,
                "bias": bias,
            }
        )
    return in_maps


_NC_CACHE = {}


def _get_nc():
    if "nc" not in _NC_CACHE:
        _NC_CACHE["nc"] = build_nc()
    return _NC_CACHE["nc"]


def _run(inputs, **spmd_kwargs):
    nc = _get_nc()
    in_maps = pack_inputs(inputs)
    out = np.empty((B, D), np.float32)
    res = run_bass_kernel_spmd(nc, in_maps, list(range(N_CORES)), **spmd_kwargs)
    for c in range(N_CORES):
        out[c * B_SHARD : (c + 1) * B_SHARD, :] = res.results[c]["out"].T
    return out, [res]


def kernel(**inputs):
    out, _ = _run(inputs)
    return out


# revision 5
# speedup vs baseline: 1.0024x; 1.0024x over previous
"""GRU cell kernel for Trainium2, 8-core data-parallel, single dispatch,
partial-fp8 matmuls.

Strategy
--------
Data-parallel on batch across 8 cores; each core processes its full
2048-row shard in ONE dispatch.  All on-chip compute happens in
transposed space ([hidden, batch]):

    r^T = sigmoid(W_r @ x^T + U_r @ h^T + b_r)      <- fp8 DoubleRow
    u^T = sigmoid(W_u @ x^T + U_u @ h^T + b_u)      <- bf16
    c^T = tanh   (W @ x^T   + U @ (h.r)^T + b_c)    <- W@x bf16, U@hr fp8
    o^T = h^T + u^T * (c^T - h^T)

fp8e4(DoubleRow) matmuls run 2x the bf16 rate on real HW (measured).
Full-fp8 busts the 2e-2 error budget (3.2e-2 measured on the real
inputs), but the reset-gate path is attenuated by sigmoid' and U-row
norms (rel 4.0e-3) and the U@hr matmul by tanh'/u (combined rel
1.2e-2), so exactly those run in fp8.

Measured-HW tensor-engine facts baked into the schedule:
  - ldweights serializes with the matmul (~44ns per 128-row stationary);
    processing the two chunks of a PAIR back-to-back under one
    stationary halves it.  fp8 DoubleRow weight loads are ~hidden.
  - Each DMA instruction occupies the single HWDGE generator ~625ns
    regardless of size -> consolidated loads (one per (matrix, out-tile)
    and per x/h chunk; 65 loads total).

Weights + x/h are fully SBUF-resident in distinct tile slots, so no DMA
ever writes a recycled slot -- this toolchain's DMA descriptors encode
exactly ONE sync wait, so any DMA needing a cross-engine WAR/RAW wait
on top of its queue-FIFO wait fails walrus codegen.  Loads carry only
queue waits on the SP HWDGE queue; output stores ride the Activation
engine's HWDGE queue (drained of loads by then), so their single RAW
wait fits.  Biases ride the ScalarE activation (per-partition bias)
which also evicts PSUM and casts in the same instruction.

Per-partition SBUF budget (usable ~208 KiB):
  weights 48(bf16)+24(fp8) + x 32+16 + h 32+16 + hr8 8 + u 16
  + r/c/out ~11 = ~203 KiB.
"""

import sys

sys.path.insert(0, "/opt/trn_rl_repo")

import numpy as np
import ml_dtypes
from contextlib import ExitStack

import concourse.bass as bass
import concourse.bacc as bacc
import concourse.mybir as mybir
from concourse import tile
from concourse.bass_utils import run_bass_kernel_spmd

BF16 = mybir.dt.bfloat16
FP8 = mybir.dt.float8e4
F32 = mybir.dt.float32
AF = mybir.ActivationFunctionType
DR = mybir.MatmulPerfMode.DoubleRow

N_CORES = 8
B = 16384
D = 1024  # IN == H
N_ROUNDS = 1
B_SHARD = B // N_CORES  # 2048 rows per core
W = 512  # chunk width (one fp32 PSUM bank)
PAIR = 2  # chunks per stationary-reuse group


def build_nc(d=D, b_shard=B_SHARD, w=W):
    """Build the SPMD per-core Bass program.

    wts8 packs fp8 mats [W_r, U_r, U]; wtsb packs bf16 mats [W_u, U_u, W].
    Bias columns: [r: 0..nh) [u: nh..2nh) [c: 2nh..3nh).
    """
    nk = d // 128
    nh = d // 128
    nch = b_shard // w

    nc = bacc.Bacc("TRN2", target_bir_lowering=False)
    xt = nc.dram_tensor("xt", [d, b_shard], BF16, kind="ExternalInput")
    ht = nc.dram_tensor("ht", [d, b_shard], BF16, kind="ExternalInput")
    xt8 = nc.dram_tensor("xt8", [d, b_shard], FP8, kind="ExternalInput")
    ht8 = nc.dram_tensor("ht8", [d, b_shard], FP8, kind="ExternalInput")
    # wts*[mat, j, p, k*128+m] = M.T[k*128+p, j*128+m]
    wtsb = nc.dram_tensor("wtsb", [3, nh, 128, nk * 128], BF16, kind="ExternalInput")
    wts8 = nc.dram_tensor("wts8", [3, nh, 128, nk * 128], FP8, kind="ExternalInput")
    bias = nc.dram_tensor("bias", [128, 3 * nh], F32, kind="ExternalInput")
    out = nc.dram_tensor("out", [d, b_shard], F32, kind="ExternalOutput")

    with tile.TileContext(nc) as tc, ExitStack() as ctx:
        xp = ctx.enter_context(tc.tile_pool(name="xp", bufs=nch))
        hp = ctx.enter_context(tc.tile_pool(name="hp", bufs=nch))
        xp8 = ctx.enter_context(tc.tile_pool(name="xp8", bufs=nch))
        hp8 = ctx.enter_context(tc.tile_pool(name="hp8", bufs=nch))
        up = ctx.enter_context(tc.tile_pool(name="up", bufs=PAIR * nh))
        hr8p = ctx.enter_context(tc.tile_pool(name="hr8p", bufs=PAIR))
        cp = ctx.enter_context(tc.tile_pool(name="cp", bufs=3))
        rp = ctx.enter_context(tc.tile_pool(name="rp", bufs=2))
        wpb = ctx.enter_context(tc.tile_pool(name="wpb", bufs=3 * nh))
        wp8 = ctx.enter_context(tc.tile_pool(name="wp8", bufs=3 * nh))
        bp = ctx.enter_context(tc.tile_pool(name="bp", bufs=1))
        op = ctx.enter_context(tc.tile_pool(name="op", bufs=3))
        pp = ctx.enter_context(tc.tile_pool(name="pp", bufs=8, space="PSUM"))

        xts = [None] * nch  # [chunk] -> [128, nk, w] bf16
        hts = [None] * nch
        x8s = [None] * nch  # [chunk] -> [128, nk, w] fp8
        h8s = [None] * nch
        wb = [[None] * nh for _ in range(3)]  # [W_u, U_u, W] bf16 [128, nk, 128]
        w8 = [[None] * nh for _ in range(3)]  # [W_r, U_r, U] fp8 [128, nk, 128]

        def load_chunk(pool, src, store, c, dt):
            t = pool.tile([128, nk, w], dt, name=f"{src.name}tile")
            nc.sync.dma_start(
                t, src[:, c * w : (c + 1) * w].rearrange("(k p) n -> p k n", p=128)
            )
            store[c] = t

        def load_w(src, store, mat, j, dt):
            t = (wp8 if dt is FP8 else wpb).tile([128, nk, 128], dt, name="wtile")
            nc.sync.dma_start(t, src[mat, j, :, :].rearrange("p (k m) -> p k m", m=128))
            store[mat][j] = t

        # DMA issue order = consumption order.  R pair-0 needs only the fp8
        # weights + pair-0 fp8 x/h; U pair-0 (starting ~30us in) needs bf16
        # x/h + Wu/Uu, so those come right after; C-phase weights and the
        # second pair's chunks trail.
        # j0 of the R phase consumes BOTH chunks of pair 0 (pair-interleaved
        # matmuls), x-part first: x8 for both chunks, then Ur0/h8.
        load_w(wts8, w8, 0, 0, FP8)
        for c in range(PAIR):
            load_chunk(xp8, xt8, x8s, c, FP8)
        for j in range(1, 4):
            load_w(wts8, w8, 0, j, FP8)
        for c in range(PAIR):
            load_chunk(hp8, ht8, h8s, c, FP8)
        for j in range(4):
            load_w(wts8, w8, 1, j, FP8)
        btile = bp.tile([128, 3 * nh], F32, name="btile")
        nc.sync.dma_start(btile, bias[:, :])
        for j in range(4, nh):
            load_w(wts8, w8, 0, j, FP8)
            load_w(wts8, w8, 1, j, FP8)
        for c in range(PAIR):
            load_chunk(xp, xt, xts, c, BF16)
            load_chunk(hp, ht, hts, c, BF16)
        for j in range(nh):
            load_w(wtsb, wb, 0, j, BF16)
            load_w(wtsb, wb, 1, j, BF16)
        for j in range(nh):
            load_w(wtsb, wb, 2, j, BF16)
        for j in range(nh):
            load_w(wts8, w8, 2, j, FP8)
        for c in range(PAIR, nch):
            load_chunk(xp8, xt8, x8s, c, FP8)
            load_chunk(hp8, ht8, h8s, c, FP8)
            load_chunk(xp, xt, xts, c, BF16)
            load_chunk(hp, ht, hts, c, BF16)

        def pair_matmuls_bf16(cs, j, mat_x, mov_x, mat_h, mov_h):
            """One PSUM bank per chunk of the pair; stationary reused across
            the pair's chunks (halves the serialized ldweights cost)."""
            pss = [pp.tile([128, w], F32, name="ps") for _ in cs]
            for mi, (mat, movs) in enumerate(((mat_x, mov_x), (mat_h, mov_h))):
                for k in range(nk):
                    lhsT = wb[mat][j][:, k, :]
                    for ci, c in enumerate(cs):
                        nc.tensor.matmul(
                            pss[ci], lhsT, movs[c][:, k, :],
                            start=(mi == 0 and k == 0),
                            stop=(mi == 1 and k == nk - 1),
                        )
            return pss

        for p in range(nch // PAIR):
            cs = list(range(p * PAIR, (p + 1) * PAIR))

            # R phase (fp8 DoubleRow): r = sigmoid(.); hr8 = fp8(h * r)
            hr8 = {}
            for c in cs:
                hr8[c] = hr8p.tile([128, nk, w], FP8, name="hr8tile")

            def r_mats(pss, j, mat, movs, is_start, is_stop):
                for kk in range(0, nk, 2):
                    lhsT = w8[mat][j][:, kk : kk + 2, :]
                    for ci, c in enumerate(cs):
                        nc.tensor.matmul(
                            pss[ci], lhsT, movs[c][:, kk : kk + 2, :],
                            start=(is_start and kk == 0),
                            stop=(is_stop and kk == nk - 2),
                            perf_mode=DR,
                        )

            def r_tail(pss, j):
                for ci, c in enumerate(cs):
                    rtile = rp.tile([128, w], BF16, name="rtile")
                    nc.scalar.activation(
                        rtile, pss[ci], AF.Sigmoid, bias=btile[:, j : j + 1]
                    )
                    nc.vector.tensor_mul(hr8[c][:, j, :], hts[c][:, j, :], rtile)

            if p == 0:
                # warm-up: run the j0..j3 x-parts back to back (4 pair-groups
                # = all 8 PSUM banks open) so the h8 chunk + U_r loads queued
                # behind x8 land while the PE is busy
                warm = range(4)
                ps01 = {}
                for j in warm:
                    ps01[j] = [pp.tile([128, w], F32, name="ps") for _ in cs]
                    r_mats(ps01[j], j, 0, x8s, True, False)
                for j in warm:
                    r_mats(ps01[j], j, 1, h8s, False, True)
                    r_tail(ps01[j], j)
                j_start = len(warm)
            else:
                j_start = 0
            for j in range(j_start, nh):
                pss = [pp.tile([128, w], F32, name="ps") for _ in cs]
                r_mats(pss, j, 0, x8s, True, False)
                r_mats(pss, j, 1, h8s, False, True)
                r_tail(pss, j)

            # U phase (bf16)
            us = {}
            for j in range(nh):
                pss = pair_matmuls_bf16(cs, j, 0, xts, 1, hts)
                for ci, c in enumerate(cs):
                    util = up.tile([128, w], BF16, name="utile")
                    nc.scalar.activation(
                        util, pss[ci], AF.Sigmoid, bias=btile[:, nh + j : nh + j + 1]
                    )
                    us[(c, j)] = util

            # C phase: W@x in bf16, U@hr in fp8 DoubleRow; OUT fused per (j,c)
            for j in range(nh):
                pss = [pp.tile([128, w], F32, name="ps") for _ in cs]
                for k in range(nk):
                    lhsT = wb[2][j][:, k, :]
                    for ci, c in enumerate(cs):
                        nc.tensor.matmul(
                            pss[ci], lhsT, xts[c][:, k, :],
                            start=(k == 0), stop=False,
                        )
                for kk in range(0, nk, 2):
                    lhsT = w8[2][j][:, kk : kk + 2, :]
                    for ci, c in enumerate(cs):
                        nc.tensor.matmul(
                            pss[ci], lhsT, hr8[c][:, kk : kk + 2, :],
                            start=False, stop=(kk == nk - 2),
                            perf_mode=DR,
                        )
                for ci, c in enumerate(cs):
                    ctile = cp.tile([128, w], BF16, name="ctile")
                    nc.scalar.activation(
                        ctile, pss[ci], AF.Tanh, bias=btile[:, 2 * nh + j : 2 * nh + j + 1]
                    )
                    tb = cp.tile([128, w], BF16, name="tbtile")
                    nc.vector.tensor_sub(tb, ctile, hts[c][:, j, :])
                    nc.vector.tensor_mul(tb, us[(c, j)], tb)
                    t = op.tile([128, w], F32, name="ttile")
                    nc.vector.tensor_add(t, tb, hts[c][:, j, :])
                    # store via the Activation engine's HWDGE queue: single
                    # RAW wait on t (loads use the SP queue, so no FIFO
                    # backpressure), avoiding the Pool SWDGE's ~1us
                    # descriptor-generation that serialized the tail.
                    nc.scalar.dma_start(
                        out[j * 128 : (j + 1) * 128, c * w : (c + 1) * w], t
                    )

    nc.compile()
    return nc


def pack_inputs(inputs, d=D, b_shard=B_SHARD, n_shards=N_CORES):
    """Host-side shard + transpose + cast. Returns per-shard input maps."""
    nk = d // 128
    nh = d // 128
    x = np.asarray(inputs["x_t"], np.float32)
    h = np.asarray(inputs["h_prev"], np.float32)

    def pack_w(m, dt):
        mt = np.asarray(m, np.float32).T.astype(dt)  # [in, out]
        # [j, p, k*128+m']
        return mt.reshape(nk, 128, nh, 128).transpose(2, 1, 0, 3).reshape(nh, 128, nk * 128)

    wtsb = np.stack(
        [pack_w(inputs[nm], ml_dtypes.bfloat16) for nm in ("W_u", "U_u", "W")]
    )
    wts8 = np.stack(
        [pack_w(inputs[nm], ml_dtypes.float8_e4m3fn) for nm in ("W_r", "U_r", "U")]
    )

    b_r = np.asarray(inputs["b_Wr"], np.float32) + np.asarray(inputs["b_Ur"], np.float32)
    b_u = np.asarray(inputs["b_Wu"], np.float32) + np.asarray(inputs["b_Uu"], np.float32)
    b_c = np.asarray(inputs["b_W"], np.float32) + np.asarray(inputs["b_U"], np.float32)
    bias = np.concatenate(
        [bb.reshape(nh, 128).T for bb in (b_r, b_u, b_c)], axis=1
    ).astype(np.float32)  # [128, 3*nh]

    in_maps = []
    for s in range(n_shards):
        rows = slice(s * b_shard, (s + 1) * b_shard)
        xT = np.ascontiguousarray(x[rows].T)
        hT = np.ascontiguousarray(h[rows].T)
        in_maps.append(
            {
                "xt": xT.astype(ml_dtypes.bfloat16),
                "ht": hT.astype(ml_dtypes.bfloat16),
                "xt8": xT.astype(ml_dtypes.float8_e4m3fn),
                "ht8": hT.astype(ml_dtypes.float8_e4m3fn),
                "wtsb": wtsb,
                "wts8": wts8,
                "bias": bias,
            }
        )
    return in_maps


_NC_CACHE = {}


def _get_nc():
    if "nc" not in _NC_CACHE:
        _NC_CACHE["nc"] = build_nc()
    return _NC_CACHE["nc"]


def _run(inputs, **spmd_kwargs):
    nc = _get_nc()
    in_maps = pack_inputs(inputs)
    out = np.empty((B, D), np.float32)
    res = run_bass_kernel_spmd(nc, in_maps, list(range(N_CORES)), **spmd_kwargs)
    for c in range(N_CORES):
        out[c * B_SHARD : (c + 1) * B_SHARD, :] = res.results[c]["out"].T
    return out, [res]


def kernel(**inputs):
    out, _ = _run(inputs)
    return out
